# revision 11
# baseline (speedup 1.0000x reference)
"""Single-layer transformer encoder forward (embed + causal attention + LM head
+ CE loss) on 8 Trainium2 NeuronCores.

Sharding: attention is tensor-parallel over heads (2 heads/core), the LM head is
tensor-parallel over vocab (4000 cols/core). The attention output is AllGathered
on-device; softmax denominators for the loss are computed on-device per vocab
shard and combined on the host.
"""

import sys

sys.path.insert(0, "/opt/trn_rl_repo")

from contextlib import ExitStack

import ml_dtypes
import numpy as np

import concourse.bass as bass
import concourse.bacc as bacc
import concourse.mybir as mybir
import concourse.tile as tile
from concourse.bass_utils import run_bass_kernel_spmd
from concourse.masks import make_identity

P = 128
B, T = 2, 2048
E = 1024
NH = 16
V = 32000
NCORES = 8

F32 = mybir.dt.float32
F32R = mybir.dt.float32r
BF16 = mybir.dt.bfloat16

# matmul dtypes: attention path and lm-head path
ATT_DT = BF16
LM_DT = BF16

_NP_DT = {BF16: ml_dtypes.bfloat16, F32: np.float32}


def _dims():
    ntok = B * T
    hd = E // NH
    hpc = NH // NCORES
    assert hpc * hd == P, "per-core head slice must be 128 wide"
    vs = V // NCORES
    nvc = 8
    vc = vs // nvc
    assert vc * nvc == vs
    return dict(NTOK=ntok, NIT=ntok // P, ET=E // P, HD=hd, HPC=hpc,
                VS=vs, VC=vc, NVC=nvc, NJ=T // P)


def _mm(ap):
    """AP wrapper for matmul operands: full-speed relaxed fp32 when fp32."""
    if ap.dtype == F32:
        return ap.bitcast(F32R)
    return ap


def build_program():
    d = _dims()
    NTOK, NIT, ET, HD, HPC = d["NTOK"], d["NIT"], d["ET"], d["HD"], d["HPC"]
    VS, VC, NVC, NJ = d["VS"], d["VC"], d["NVC"], d["NJ"]

    nc = bacc.Bacc("TRN2", target_bir_lowering=False, debug=False, num_devices=NCORES)

    tok_emb = nc.dram_tensor("tok_emb", [V, E], F32, kind="ExternalInput").ap()
    pos = nc.dram_tensor("pos", [T, E], F32, kind="ExternalInput").ap()
    idx = nc.dram_tensor("idx", [P, NIT], mybir.dt.int32, kind="ExternalInput").ap()
    wq = nc.dram_tensor("wq", [E, P], ATT_DT, kind="ExternalInput").ap()
    wk = nc.dram_tensor("wk", [E, P], ATT_DT, kind="ExternalInput").ap()
    wv = nc.dram_tensor("wv", [E, P], ATT_DT, kind="ExternalInput").ap()
    wlm = nc.dram_tensor("wlm", [E, VS], LM_DT, kind="ExternalInput").ap()
    blm = nc.dram_tensor("blm", [1, VS], F32, kind="ExternalInput").ap()
    maskadd = nc.dram_tensor("maskadd", [P, P], F32, kind="ExternalInput").ap()

    logits_out = nc.dram_tensor("logits_out", [NTOK, VS], F32, kind="ExternalOutput").ap()
    sumexp_out = nc.dram_tensor("sumexp_out", [P, NIT], F32, kind="ExternalOutput").ap()

    otp = nc.dram_tensor("otp", [P, NTOK], LM_DT).ap()
    otf = nc.dram_tensor("otf", [NCORES * P, NTOK], LM_DT, addr_space="Shared").ap()

    with tile.TileContext(nc) as tc, ExitStack() as ctx:
        const = ctx.enter_context(tc.tile_pool(name="const", bufs=1))
        persist = ctx.enter_context(tc.tile_pool(name="persist", bufs=1))

        # ---- constants ----
        wq_sb = const.tile([P, ET, P], ATT_DT)
        wk_sb = const.tile([P, ET, P], ATT_DT)
        wv_sb = const.tile([P, ET, P], ATT_DT)
        nc.sync.dma_start(wq_sb[:], wq.rearrange("(ko p) m -> p ko m", p=P))
        nc.sync.dma_start(wk_sb[:], wk.rearrange("(ko p) m -> p ko m", p=P))
        nc.sync.dma_start(wv_sb[:], wv.rearrange("(ko p) m -> p ko m", p=P))
        mask_sb = const.tile([P, P], F32)
        nc.sync.dma_start(mask_sb[:], maskadd[:])
        idx_sb = const.tile([P, NIT], mybir.dt.int32)
        nc.sync.dma_start(idx_sb[:], idx[:])
        ident_f = const.tile([P, P], F32)
        make_identity(nc, ident_f[:])
        ident_a = const.tile([P, P], ATT_DT)
        make_identity(nc, ident_a[:])

        # Absorb constant-DMA waits onto the DVE clock so downstream DVE
        # TensorTensor ops (1 wait slot in the ISA encoding) never need them.
        touch_t = const.tile([1, 1], F32)

        def _touch(ap):
            nc.vector.tensor_copy(touch_t[:1, :1], ap)

        _touch(mask_sb[:1, :1])

        # ---- persistent activations ----
        qT_sb = persist.tile([P, NTOK], ATT_DT)   # [2*HD, tokens]
        kT_sb = persist.tile([P, NTOK], ATT_DT)
        v_sb = persist.tile([P, NIT, P], ATT_DT)  # [tok%128, tile, 2*HD]
        ot_sb = persist.tile([P, NTOK], LM_DT)    # [2*HD, tokens] attention out^T

        # ---- phase B: embedding gather + transpose to hT ----
        with ExitStack() as pctx:
            hpool = pctx.enter_context(tc.tile_pool(name="hT", bufs=1))
            embw = pctx.enter_context(tc.tile_pool(name="embw", bufs=3))
            tps = pctx.enter_context(tc.tile_pool(name="tps", bufs=2, space="PSUM"))

            hT_sb = hpool.tile([P, ET, NTOK], ATT_DT)
            pos_sb = hpool.tile([P, NJ, E], F32)
            nc.sync.dma_start(pos_sb[:], pos.rearrange("(o p) e -> p o e", p=P))
            _touch(pos_sb[:1, 0, :1])
            for it in range(NIT):
                h_t = embw.tile([P, E], F32, tag="h")
                nc.gpsimd.indirect_dma_start(
                    out=h_t[:], out_offset=None, in_=tok_emb[:],
                    in_offset=bass.IndirectOffsetOnAxis(ap=idx_sb[:, it : it + 1], axis=0),
                )
                nc.vector.tensor_tensor(
                    out=h_t[:], in0=h_t[:], in1=pos_sb[:, it % NJ, :], op=mybir.AluOpType.add
                )
                for e in range(ET):
                    t_ps = tps.tile([P, P], F32, tag="t")
                    nc.tensor.transpose(t_ps[:], h_t[:, e * P : (e + 1) * P], ident_f[:])
                    nc.vector.tensor_copy(hT_sb[:, e, it * P : (it + 1) * P], t_ps[:])

            # ---- phase C: QKV projections ----
            CH = 512
            for ch in range(NTOK // CH):
                for w_sb, dst in ((wq_sb, qT_sb), (wk_sb, kT_sb)):
                    ps = tps.tile([P, CH], F32, tag="qk")
                    for e in range(ET):
                        nc.tensor.matmul(
                            ps[:], lhsT=_mm(w_sb[:, e, :]),
                            rhs=_mm(hT_sb[:, e, ch * CH : (ch + 1) * CH]),
                            start=(e == 0), stop=(e == ET - 1),
                        )
                    nc.vector.tensor_copy(dst[:, ch * CH : (ch + 1) * CH], ps[:])
            for it in range(NIT):
                ps = tps.tile([P, P], F32, tag="v")
                for e in range(ET):
                    nc.tensor.matmul(
                        ps[:], lhsT=_mm(hT_sb[:, e, it * P : (it + 1) * P]),
                        rhs=_mm(wv_sb[:, e, :]),
                        start=(e == 0), stop=(e == ET - 1),
                    )
                nc.vector.tensor_copy(v_sb[:, it, :], ps[:])

        # ---- phase D: causal attention for this core's heads ----
        with ExitStack() as pctx:
            attw = pctx.enter_context(tc.tile_pool(name="attw", bufs=2))
            aps = pctx.enter_context(tc.tile_pool(name="aps", bufs=2, space="PSUM"))
            for b in range(B):
                for hl in range(HPC):
                    q0 = HD * hl
                    for it in range(NJ):
                        nj = it + 1
                        jmax = nj * P
                        iofs = b * T + it * P
                        p_t = attw.tile([P, T], ATT_DT, tag="P")
                        rs = attw.tile([P, 4], F32, tag="rs")
                        nch = (jmax + 511) // 512
                        for ch in range(nch):
                            w = min(512, jmax - ch * 512)
                            s_ps = aps.tile([P, 512], F32, tag="S")
                            nc.tensor.matmul(
                                s_ps[:, :w],
                                lhsT=_mm(qT_sb[q0 : q0 + HD, iofs : iofs + P]),
                                rhs=_mm(kT_sb[q0 : q0 + HD, b * T + ch * 512 : b * T + ch * 512 + w]),
                                start=True, stop=True,
                            )
                            dg = it * P - ch * 512  # diagonal block offset in chunk
                            if 0 <= dg < w:
                                nc.vector.tensor_tensor(
                                    out=s_ps[:, dg : dg + P], in0=s_ps[:, dg : dg + P],
                                    in1=mask_sb[:], op=mybir.AluOpType.add,
                                )
                            nc.scalar.activation(
                                p_t[:, ch * 512 : ch * 512 + w], s_ps[:, :w],
                                mybir.ActivationFunctionType.Exp,
                                scale=1.0 / (HD ** 0.5),
                                accum_out=rs[:, ch : ch + 1],
                            )
                        rtot = attw.tile([P, 1], F32, tag="rt")
                        nc.vector.reduce_sum(rtot[:, :1], rs[:, :nch], axis=mybir.AxisListType.X)
                        recip = attw.tile([P, 1], F32, tag="rc")
                        nc.vector.reciprocal(recip[:, :1], rtot[:, :1])
                        nc.vector.tensor_scalar_mul(p_t[:, :jmax], p_t[:, :jmax], recip[:, :1])
                        pt_t = attw.tile([P, NJ, P], ATT_DT, tag="PT")
                        for jt in range(nj):
                            pt_ps = aps.tile([P, P], ATT_DT, tag="PTp")
                            nc.tensor.transpose(pt_ps[:], p_t[:, jt * P : (jt + 1) * P], ident_a[:])
                            nc.vector.tensor_copy(pt_t[:, jt, :], pt_ps[:])
                        ot_ps = aps.tile([HD, P], F32, tag="OT")
                        for jt in range(nj):
                            nc.tensor.matmul(
                                ot_ps[:],
                                lhsT=_mm(v_sb[:, b * NJ + jt, q0 : q0 + HD]),
                                rhs=_mm(pt_t[:, jt, :]),
                                start=(jt == 0), stop=(jt == nj - 1),
                            )
                        nc.vector.tensor_copy(ot_sb[q0 : q0 + HD, iofs : iofs + P], ot_ps[:])

        # ---- phase E: AllGather attention output ----
        nc.sync.dma_start(otp[:], ot_sb[:])
        nc.gpsimd.collective_compute(
            "AllGather", mybir.AluOpType.bypass,
            replica_groups=[list(range(NCORES))],
            ins=[otp[:]], outs=[otf[:]],
        )

        # ---- phase F: LM head on vocab shard + softmax partials ----
        with ExitStack() as pctx:
            lmw = pctx.enter_context(tc.tile_pool(name="lmw", bufs=1))
            lmd = pctx.enter_context(tc.tile_pool(name="lmd", bufs=2))
            lps = pctx.enter_context(tc.tile_pool(name="lps", bufs=3, space="PSUM"))

            ot_full = lmw.tile([P, ET, NTOK], LM_DT)
            nc.sync.dma_start(ot_full[:], otf.rearrange("(ko p) i -> p ko i", p=P))
            bias_sb = lmw.tile([P, VS], F32)
            nc.sync.dma_start(bias_sb[:], blm[:1, :].partition_broadcast(P))
            _touch(bias_sb[:1, :1])
            separts = lmw.tile([P, NIT, NVC], F32)

            for n in range(NVC):
                wc = lmd.tile([P, ET, VC], LM_DT, tag="wc")
                nc.sync.dma_start(
                    wc[:], wlm[:, n * VC : (n + 1) * VC].rearrange("(ko p) m -> p ko m", p=P)
                )
                for it in range(NIT):
                    l_ps = lps.tile([P, VC], F32, tag="L")
                    for e in range(ET):
                        nc.tensor.matmul(
                            l_ps[:], lhsT=_mm(ot_full[:, e, it * P : (it + 1) * P]),
                            rhs=_mm(wc[:, e, :]),
                            start=(e == 0), stop=(e == ET - 1),
                        )
                    l_sb = lmd.tile([P, VC], F32, tag="ls")
                    nc.vector.tensor_tensor(
                        out=l_sb[:], in0=l_ps[:], in1=bias_sb[:, n * VC : (n + 1) * VC],
                        op=mybir.AluOpType.add,
                    )
                    e_sb = lmd.tile([P, VC], F32, tag="es")
                    nc.scalar.activation(
                        e_sb[:], l_sb[:], mybir.ActivationFunctionType.Exp,
                        accum_out=separts[:, it, n : n + 1],
                    )
                    nc.sync.dma_start(
                        logits_out[it * P : (it + 1) * P, n * VC : (n + 1) * VC], l_sb[:]
                    )
            se_sb = lmw.tile([P, NIT], F32)
            nc.vector.reduce_sum(se_sb[:], separts[:], axis=mybir.AxisListType.X)
            nc.sync.dma_start(sumexp_out[:], se_sb[:])

    nc.compile()
    return nc


def make_in_maps(x, tok_emb, pos_emb, Wq, Wk, Wv, W_lm, b_lm):
    d = _dims()
    NTOK, NIT, VS = d["NTOK"], d["NIT"], d["VS"]
    att_np = _NP_DT[ATT_DT]
    lm_np = _NP_DT[LM_DT]

    idx_np = np.ascontiguousarray(
        np.asarray(x).reshape(NTOK).astype(np.int32).reshape(NIT, P).T
    )
    maskadd_np = ((np.tril(np.ones((P, P), np.float32)) - 1.0) * 1e30).astype(np.float32)

    tok_emb = np.ascontiguousarray(np.asarray(tok_emb, dtype=np.float32))
    pos_emb = np.ascontiguousarray(np.asarray(pos_emb, dtype=np.float32))[:T]
    Wq = np.asarray(Wq, dtype=np.float32)
    Wk = np.asarray(Wk, dtype=np.float32)
    Wv = np.asarray(Wv, dtype=np.float32)
    W_lm = np.asarray(W_lm, dtype=np.float32)
    b_lm = np.asarray(b_lm, dtype=np.float32)

    in_maps = []
    for c in range(NCORES):
        hs = slice(c * P, (c + 1) * P)        # head-dim slice
        vs = slice(c * VS, (c + 1) * VS)      # vocab slice
        in_maps.append({
            "tok_emb": tok_emb,
            "pos": pos_emb,
            "idx": idx_np,
            "wq": np.ascontiguousarray(Wq[:, hs]).astype(att_np),
            "wk": np.ascontiguousarray(Wk[:, hs]).astype(att_np),
            "wv": np.ascontiguousarray(Wv[:, hs]).astype(att_np),
            "wlm": np.ascontiguousarray(W_lm[:, vs]).astype(lm_np),
            "blm": np.ascontiguousarray(b_lm[vs]).reshape(1, VS),
            "maskadd": maskadd_np,
        })
    return in_maps


def assemble(results, target):
    """results: list of per-core dicts with logits_out / sumexp_out."""
    d = _dims()
    NTOK = d["NTOK"]
    logits_full = np.concatenate(
        [np.asarray(results[c]["logits_out"]) for c in range(NCORES)], axis=1
    )
    sumexp = np.zeros(NTOK, np.float64)
    for c in range(NCORES):
        sumexp += np.asarray(results[c]["sumexp_out"]).T.reshape(NTOK).astype(np.float64)
    logz = np.log(sumexp)
    tgt = logits_full[np.arange(NTOK), np.asarray(target).reshape(NTOK).astype(np.int64)]
    loss = np.float32(np.mean(logz - tgt.astype(np.float64)))
    return logits_full, loss


_NC = None
LAST_EXEC_NS = None
LAST_RES = None


def kernel(x, target, tok_emb, pos_emb, Wq, Wk, Wv, W_lm, b_lm, **_):
    global _NC, LAST_EXEC_NS, LAST_RES
    if _NC is None:
        _NC = build_program()
    in_maps = make_in_maps(x, tok_emb, pos_emb, Wq, Wk, Wv, W_lm, b_lm)
    res = run_bass_kernel_spmd(_NC, in_maps, list(range(NCORES)))
    LAST_RES = res
    LAST_EXEC_NS = res.exec_time_ns
    return assemble(res.results, target)
